# revision 44
# baseline (speedup 1.0000x reference)
"""Causal self-attention (RoPE quirk variant) on 8 Trainium2 NeuronCores.

Reference computation (B=2, T=1024, C=2048, H=64 heads, hd=32):
  qkv = x @ w_attn; split q,k,v; RoPE(dim=n_head quirk) on q,k;
  causal softmax attention; y @ w_proj.

Sharding: 8 cores = 2 batches x 4 head-quarter shards (16 heads / core).
Each core computes attention for its 16 heads on its batch and a partial
output projection (its 512 channels of the 2048-channel contraction);
the host sums the 4 partials per batch.

Device layout notes:
  * Everything "transposed": qkv^T [chan, T] so the attention matmuls
    (contraction over hd on partitions) and the out-projection
    (contraction over channels on partitions) need no transposes.
  * All HBM traffic is bf16 (x, w_attn, w_proj, out) - halves DMA time;
    matmuls accumulate fp32 in PSUM so the overall error stays ~0.6%.
  * rotate_half is a signed permutation matmul on PE (a partition
    pair-swap DMA gets only half bandwidth at 64 partitions and its
    queue latency sat on the S critical path).
  * Scores are built as S^T [T_k, T_q]: 4 heads row-strip-packed, 2
    heads per 2-bank PSUM tile; the causal mask is a pre-exp -1e30 add
    on the PSUM scores (on the S->exp edge, hidden under ACT's queue,
    instead of the exp->PV critical edge); one exp per head pair on
    ACT; denominators via a ones-matrix matmul whose 4 col-strip
    matmuls pack with the PV quads (measured: col-strip quads start
    within 10 ns; row-strip S quads serialize on the PSUM write port);
    1/z = exp(-ln z) on ACT (bass blocks the Reciprocal LUT and the
    DVE reciprocal costs 6.4 ns/col).
  * Projections accumulate half-T chains into 2 double-buffered PSUM
    banks so DVE cast drains overlap the next chain (ACT stays
    dedicated to exp).
  * Schedule shape (everything is emission-order = in-order per
    engine): warm-up matmuls against the consts tile pre-release the
    HAM clock gate (idle default 1.2 GHz) before x lands; per group
    the big attention half's S/exp blocks are woven with the group's
    v-projection halves, and the small half's blocks are woven with
    the NEXT group's q/k projection chains, so stretches of 32-row S
    matmuls (which read as low activity to the clock gate) always
    carry full-row projection matmuls; g3's small half interleaves
    with out-projection units (which run on the by-then-idle psa
    slots) block by block, tq 4-7 first.
"""

import json
import sys

sys.path.insert(0, "/opt/trn_rl_repo")

import ml_dtypes
import numpy as np

import concourse.bass as bass
import concourse.mybir as mybir
import concourse.tile as tile
from concourse.tile import add_dep_helper

F32 = mybir.dt.float32
BF16 = mybir.dt.bfloat16

B, T, C = 2, 1024, 2048
H, HD = 64, 32
SCALE = 1.0 / np.sqrt(32.0)

_PATCHED = False


def _split_excess_waits(bir_json: bytes) -> bytes:
    """The walrus build in this container encodes at most ONE sync-wait per
    instruction; Tile's wait assigner emits several. Hoist excess waits onto
    same-engine NoOps placed immediately before the instruction."""
    d = json.loads(bir_json)
    ctr = 0
    for fn in d.get("functions", []):
        for blk in fn.get("blocks", []):
            out = []
            for inst in blk.get("instructions", []):
                si = inst.get("sync_info")
                waits = (si or {}).get("on_wait") or []
                if len(waits) > 1:
                    for w in waits[:-1]:
                        out.append({
                            "name": f"WSplit-{ctr}",
                            "opcode": "NoOp",
                            "engine": inst["engine"],
                            "ins": [],
                            "outs": [],
                            "sync_info": {"on_update": [], "on_wait": [w]},
                        })
                        ctr += 1
                    si["on_wait"] = [waits[-1]]
                out.append(inst)
            blk["instructions"] = out
    return json.dumps(d).encode()


def _install_patches():
    global _PATCHED
    if _PATCHED:
        return
    import concourse.bass_utils as bu
    import concourse.bass2jax as b2j

    orig = bu.compile_bir_kernel

    def patched_compile(bir_json, tmpdir, neff_name="file.neff"):
        return orig(_split_excess_waits(bir_json), tmpdir, neff_name)

    bu.compile_bir_kernel = patched_compile
    b2j.compile_bir_kernel = patched_compile
    _PATCHED = True


def _build_bass():
    nc = bass.Bass(trn_type="TRN2")
    xT = nc.dram_tensor("xT", [128, 16, 1024], BF16, kind="ExternalInput").ap()
    wA = nc.dram_tensor("wA", [128, 12, 16, 128], BF16, kind="ExternalInput").ap()
    wP = nc.dram_tensor("wP", [128, 4, 2048], BF16, kind="ExternalInput").ap()
    # consts: cos[1024] | sin[1024] | ident[128] | perm[128] | trineg[256]
    consts = nc.dram_tensor("consts", [128, 2560], BF16, kind="ExternalInput").ap()
    out = nc.dram_tensor("out", [1024, 2048], BF16, kind="ExternalOutput").ap()
    outr = out.rearrange("(tq p) n -> tq p n", p=128)

    EXP = mybir.ActivationFunctionType.Exp
    LN = mybir.ActivationFunctionType.Ln
    COPY = mybir.ActivationFunctionType.Copy

    with tile.TileContext(nc) as tc:
        with tc.tile_pool(name="persist", bufs=1) as persist, \
             tc.tile_pool(name="ylate", bufs=1) as ylate:
            # rotated q (dim1 = group 0-3) and k (4-7), bf16 [chan, T]
            qkT = persist.tile([128, 8, 1024], BF16)
            v_sb = persist.tile([128, 8, 512], BF16)    # [T_k in blk, kb, chan]
            csts = persist.tile([128, 2560], BF16)
            nc.scalar.dma_start(csts, consts)
            cos_sb = csts[:, 0:1024]
            sin_sb = csts[:, 1024:2048]
            sin2 = sin_sb.rearrange("p (a b) -> p a b", b=512)
            id_sb = csts[:, 2048:2176]
            perm_sb = csts[:, 2176:2304]                # signed rotate_half
            # [128, 2, 128]: 0 where k<=q else -1e30, for one head pair
            trineg = csts[:, 2304:2560].rearrange("p (a b) -> p a b", b=128)
            ones_sb = persist.tile([128, 32], BF16)
            nc.vector.memset(ones_sb, 1.0)
            wp_sb = persist.tile([128, 4, 2048], BF16)  # loaded at g==2
            y_tiles = [ylate.tile([128, 1024], BF16, tag=f"y{g}", name=f"y{g}")
                       for g in range(4)]

            with tc.tile_pool(name="phA", bufs=1) as xpool, \
                 tc.tile_pool(name="wstream", bufs=3) as wsp:
                xt = xpool.tile([128, 16, 1024], BF16)
                # prefetch the first weight chunk ahead of the x bulk so the
                # first projection matmul isn't stuck behind the x DMA.
                # The sync engine's bulk queue has the fastest ramp-up;
                # scalar/gpsimd bulk queues start ~3us later and drain
                # slower (measured), so everything big goes on sync.
                wt_first = wsp.tile([128, 16, 128], BF16, tag="wa", name="wt_first")
                wt_k0 = wsp.tile([128, 16, 128], BF16, tag="wa", name="wt_k0")
                wt_v0 = wsp.tile([128, 16, 128], BF16, tag="wa", name="wt_v0")
                nc.sync.dma_start(wt_first, wA[:, 0])
                for kc in range(8):
                    nc.sync.dma_start(xt[:, 2 * kc:2 * kc + 2, :],
                                      xT[:, 2 * kc:2 * kc + 2, :])
                    if kc == 0:
                        nc.sync.dma_start(wt_k0, wA[:, 4])
                    elif kc == 1:
                        nc.sync.dma_start(wt_v0, wA[:, 8])

                # ---- per-group: project q,k -> RoPE -> project v -> attn ----
                with tc.tile_pool(name="esp", bufs=10) as esp, \
                     tc.tile_pool(name="qtp", bufs=2) as qtp, \
                     tc.tile_pool(name="zp", bufs=2) as zp, \
                     tc.tile_pool(name="phD", bufs=2) as phd, \
                     tc.tile_pool(name="psA", bufs=2, space="PSUM") as psa, \
                     tc.tile_pool(name="psS", bufs=2, space="PSUM") as psS, \
                     tc.tile_pool(name="psYZ", bufs=1, space="PSUM") as psYZ:

                    def attn_sblock(g, qc, kb):
                        # scores (+ pre-exp causal mask) + exp for one 128-k
                        # block; reads only qkT, so it may be emitted before
                        # the group's v projection. The mask is a -1e30 add
                        # on the PSUM scores: it sits on the S->exp edge
                        # (hidden under ACT's queue) instead of the exp->PV
                        # critical edge.
                        q0 = qc * 512
                        k0 = kb * 128
                        n0 = max(q0, k0)
                        N = q0 + 512 - n0
                        es = esp.tile([128, 4, 512], BF16, tag="es")
                        pss = [psS.tile([128, 2, 512], F32, tag="pss",
                                        name=f"pss{g}_{qc}_{kb}_{p}")
                               for p in range(2)]
                        for h in range(4):
                            nc.tensor.matmul(
                                pss[h // 2][:, h % 2, :N],
                                qkT[32 * h:32 * h + 32, 4 + g, k0:k0 + 128],
                                qkT[32 * h:32 * h + 32, g, n0:n0 + N],
                                start=True, stop=True,
                                tile_position=(32 * h, 0))
                        if k0 >= q0:
                            for p in range(2):
                                nc.vector.tensor_add(
                                    pss[p][:, :, 0:128], pss[p][:, :, 0:128],
                                    trineg)
                        nc.scalar.activation(
                            es[:, 0:2, :N], pss[0][:, :, :N], EXP)
                        e1 = nc.scalar.activation(
                            es[:, 2:4, :N], pss[1][:, :, :N], EXP)
                        return es, e1, N, n0 - q0

                    def attn_pvz(g, qc, kb, sblk, psy, psz):
                        es, gate, N, off = sblk
                        nkb = (qc + 1) * 4
                        first = True
                        for h in range(4):
                            c0 = g * 128 + 32 * h
                            pv = nc.tensor.matmul(
                                psy[32 * h:32 * h + 32, off:512],
                                v_sb[:, kb, c0:c0 + 32],
                                es[:, h, :N],
                                start=(kb == 0), stop=(kb == nkb - 1),
                                tile_position=(0, 32 * h),
                                skip_group_check=True)
                            if first:
                                add_dep_helper(pv.ins, gate.ins,
                                               sync=True, reason="pack PV")
                                first = False
                        first = True
                        for h in range(4):
                            z = nc.tensor.matmul(
                                psz[32 * h:32 * h + 32, off:512],
                                ones_sb,
                                es[:, h, :N],
                                start=(kb == 0), stop=(kb == nkb - 1),
                                tile_position=(0, 32 * h),
                                skip_group_check=True)
                            if first:
                                add_dep_helper(z.ins, gate.ins,
                                               sync=True, reason="pack Z")
                                first = False

                    def attn_block(g, qc, kb, psy, psz):
                        attn_pvz(g, qc, kb, attn_sblock(g, qc, kb), psy, psz)

                    def attn_finish(g, q0, psy, psz):
                        # 1/z = exp(-ln(z)) on ACT (reciprocal LUT is
                        # blocked in bass; DVE reciprocal costs 6.4ns/col)
                        zinv = zp.tile([128, 512], F32, tag="zinv")
                        lnz = zp.tile([128, 512], F32, tag="lnz")
                        nc.scalar.activation(lnz, psz, LN)
                        nc.scalar.activation(zinv, lnz, EXP, scale=-1.0)
                        nc.vector.tensor_mul(y_tiles[g][:, q0:q0 + 512],
                                             psy, zinv)

                    def outproj_tq(tq):
                        # 32 independent (tq, n) units on the psa slots - no
                        # dedicated out-projection PSUM pool, so these can be
                        # interleaved with tail attention blocks
                        o_sb = phd.tile([128, 2048], BF16, tag="osb")
                        for n in range(4):
                            pso = psa.tile([128, 512], F32, tag="ps",
                                           name=f"pso{tq}_{n}")
                            for gk in range(4):
                                nc.tensor.matmul(
                                    pso, y_tiles[gk][:, tq * 128:(tq + 1) * 128],
                                    wp_sb[:, gk, n * 512:(n + 1) * 512],
                                    start=(gk == 0), stop=(gk == 3))
                            sl = slice(n * 512, (n + 1) * 512)
                            if n % 2 == 0:
                                nc.vector.tensor_copy(o_sb[:, sl], pso)
                            else:
                                nc.scalar.activation(o_sb[:, sl], pso, COPY)
                        nc.sync.dma_start(outr[tq], o_sb)

                    def rope(g, pre, swp):
                        # rotate_half is a signed permutation matmul on PE (a
                        # partition pair-swap DMA only gets half bandwidth at
                        # 64 partitions and its queue latency sat on the S
                        # critical path)
                        for j, dst in enumerate((qkT[:, g, :], qkT[:, 4 + g, :])):
                            rot = psS.tile([128, 2, 512], F32, tag="pss",
                                           name=f"rot{g}_{j}")
                            for half in (0, 1):
                                nc.tensor.matmul(
                                    rot[:, half, :], perm_sb,
                                    pre[:, j, half * 512:half * 512 + 512],
                                    start=True, stop=True)
                            nc.vector.tensor_mul(
                                swp[:, j, :].rearrange("p (a b) -> p a b", b=512),
                                rot, sin2)
                            nc.vector.tensor_mul(pre[:, j, :], pre[:, j, :], cos_sb)
                            nc.vector.tensor_add(dst, pre[:, j, :], swp[:, j, :])

                    def qk_phase(g, blocks=()):
                        # project q,k then RoPE; `blocks` are attention-block
                        # thunks of the previous group's small half, woven
                        # between the projection half-chains so the PE keeps
                        # full-row matmul activity (the HAM clock gate
                        # throttles to 1.2 GHz through stretches of 32-row
                        # S matmuls) and the exp chain stalls get filled.
                        pre = qtp.tile([128, 2, 1024], BF16, tag="pre")
                        swp = qtp.tile([128, 2, 1024], BF16, tag="swp")
                        if g == 0:
                            wtq = wt_first
                        else:
                            wtq = wsp.tile([128, 16, 128], BF16, tag="wa")
                            nc.sync.dma_start(wtq, wA[:, g])
                        wtk = wsp.tile([128, 16, 128], BF16, tag="wa")
                        nc.sync.dma_start(wtk, wA[:, 4 + g])
                        seq = [(wtq, 0, pre[:, 0, :]), (wtq, 1, pre[:, 0, :]),
                               (wtk, 0, pre[:, 1, :]), (wtk, 1, pre[:, 1, :])]
                        for i, (wt, half, dst) in enumerate(seq):
                            if i < len(blocks):
                                blocks[i]()
                            proj_half(g, wt, half, dst)
                        rope(g, pre, swp)

                    def proj_half(g, wt, half, dst):
                        ps = psa.tile([128, 512], F32, tag="ps")
                        sl = slice(half * 512, half * 512 + 512)
                        for ko in range(16):
                            nc.tensor.matmul(ps, wt[:, ko, :], xt[:, ko, sl],
                                             start=(ko == 0), stop=(ko == 15))
                        nc.vector.tensor_copy(dst[:, sl], ps)

                    def vT_range(g, vtmp, kbs):
                        # natural-layout v transpose via PE; draws from the
                        # psa pool (proj-paced) so it never queues behind
                        # exp-gated S tiles in psS
                        for kb in kbs:
                            pst = psa.tile([128, 512], F32, tag="ps",
                                           name=f"pst{g}_{kb}")[:, 0:128]
                            nc.tensor.matmul(pst,
                                             vtmp[:, kb * 128:(kb + 1) * 128],
                                             id_sb, start=True, stop=True)
                            nc.vector.tensor_copy(
                                v_sb[:, kb, g * 128:(g + 1) * 128], pst)

                    def attn1_woven(g):
                        # the group's big attention half, software-pipelined
                        # against its v projection: S/exp blocks (which need
                        # only q,k) interleave with the v-projection matmuls
                        # so the ACT exp chains hide under PE work; PV/Z
                        # follow as packed bursts once v is transposed.
                        wt = wsp.tile([128, 16, 128], BF16, tag="wa")
                        nc.sync.dma_start(wt, wA[:, 8 + g])
                        vtmp = qtp.tile([128, 1024], BF16, tag="vtmp")
                        sb = {}
                        sb[0] = attn_sblock(g, 1, 0)
                        sb[1] = attn_sblock(g, 1, 1)
                        proj_half(g, wt, 0, vtmp)
                        sb[2] = attn_sblock(g, 1, 2)
                        sb[3] = attn_sblock(g, 1, 3)
                        proj_half(g, wt, 1, vtmp)
                        sb[4] = attn_sblock(g, 1, 4)
                        vT_range(g, vtmp, range(0, 4))
                        sb[5] = attn_sblock(g, 1, 5)
                        vT_range(g, vtmp, range(4, 8))
                        sb[6] = attn_sblock(g, 1, 6)
                        sb[7] = attn_sblock(g, 1, 7)
                        psy = psYZ.tile([128, 512], F32, tag="psy")
                        psz = psYZ.tile([128, 512], F32, tag="psz")
                        for kb in range(8):
                            attn_pvz(g, 1, kb, sb[kb], psy, psz)
                        attn_finish(g, 512, psy, psz)

                    # ---- group 0: ko-outer projection. q,k,v accumulate in
                    # six parallel PSUM chains (borrowing the still-empty
                    # psS/psYZ banks) so each x chunk is consumed at its DMA
                    # arrival rate - PE is dense from the first chunk, which
                    # also releases the HAM clock gate, and all three
                    # projections finish when x lands (~21us) instead of
                    # serially at ~35us. ----
                    pre0 = qtp.tile([128, 2, 1024], BF16, tag="pre")
                    swp0 = qtp.tile([128, 2, 1024], BF16, tag="swp")
                    vtmp0 = qtp.tile([128, 1024], BF16, tag="vtmp")
                    psq = psS.tile([128, 2, 512], F32, tag="pss", name="g0q")
                    psk = psS.tile([128, 2, 512], F32, tag="pss", name="g0k")
                    psv = [psYZ.tile([128, 512], F32, tag=t, name=f"g0v{t}")
                           for t in ("psy", "psz")]
                    for ko in range(16):
                        st, sp = ko == 0, ko == 15
                        for half in (0, 1):
                            sl = slice(half * 512, half * 512 + 512)
                            nc.tensor.matmul(psq[:, half, :], wt_first[:, ko, :],
                                             xt[:, ko, sl], start=st, stop=sp,
                                             skip_group_check=True)
                            nc.tensor.matmul(psk[:, half, :], wt_k0[:, ko, :],
                                             xt[:, ko, sl], start=st, stop=sp,
                                             skip_group_check=True)
                            nc.tensor.matmul(psv[half], wt_v0[:, ko, :],
                                             xt[:, ko, sl], start=st, stop=sp,
                                             skip_group_check=True)
                    for half in (0, 1):
                        sl = slice(half * 512, half * 512 + 512)
                        nc.vector.tensor_copy(pre0[:, 0, sl], psq[:, half, :])
                        nc.scalar.activation(pre0[:, 1, sl], psk[:, half, :],
                                             COPY)
                        nc.vector.tensor_copy(vtmp0[:, sl], psv[half])
                    rope(0, pre0, swp0)
                    # g0's attention big half (v is already projected)
                    sb = {}
                    sb[0] = attn_sblock(0, 1, 0)
                    sb[1] = attn_sblock(0, 1, 1)
                    vT_range(0, vtmp0, range(0, 4))
                    sb[2] = attn_sblock(0, 1, 2)
                    sb[3] = attn_sblock(0, 1, 3)
                    vT_range(0, vtmp0, range(4, 8))
                    sb[4] = attn_sblock(0, 1, 4)
                    sb[5] = attn_sblock(0, 1, 5)
                    sb[6] = attn_sblock(0, 1, 6)
                    sb[7] = attn_sblock(0, 1, 7)
                    psy_0 = psYZ.tile([128, 512], F32, tag="psy")
                    psz_0 = psYZ.tile([128, 512], F32, tag="psz")
                    for kb in range(8):
                        attn_pvz(0, 1, kb, sb[kb], psy_0, psz_0)
                    attn_finish(0, 512, psy_0, psz_0)
                    for g in (1, 2, 3):
                        # previous group's small half woven into this
                        # group's q/k projection chains
                        gp = g - 1
                        psy0 = psYZ.tile([128, 512], F32, tag="psy")
                        psz0 = psYZ.tile([128, 512], F32, tag="psz")
                        blocks = [
                            (lambda kb=kb: attn_block(gp, 0, kb, psy0, psz0))
                            for kb in range(4)]
                        qk_phase(g, blocks)
                        attn_finish(gp, 0, psy0, psz0)
                        if g == 2:
                            nc.scalar.dma_start(wp_sb, wP)
                        attn1_woven(g)

                    # ---- tail: g3 small half interleaved with
                    # out-projection units ----
                    psy30 = psYZ.tile([128, 512], F32, tag="psy")
                    psz30 = psYZ.tile([128, 512], F32, tag="psz")
                    for kb in range(4):
                        attn_block(3, 0, kb, psy30, psz30)
                        outproj_tq(4 + kb)
                    attn_finish(3, 0, psy30, psz30)
                    for tq in (0, 1, 2, 3):
                        outproj_tq(tq)
    return nc


_NC_CACHE = None


def _host_inputs(x, pos, w_attn, w_proj):
    """Build the 8 per-core input dicts (all device traffic in bf16)."""
    x = np.asarray(x, dtype=np.float32)
    pos = np.asarray(pos, dtype=np.float32)
    w_attn = np.asarray(w_attn, dtype=np.float32)
    w_proj = np.asarray(w_proj, dtype=np.float32)

    IDENT = np.eye(128, dtype=np.float32)
    # signed rotate_half permutation: rot[2i] = -x[2i+1], rot[2i+1] = x[2i]
    PERM = np.zeros((128, 128), dtype=np.float32)
    ii = np.arange(0, 128, 2)
    PERM[ii + 1, ii] = -1.0
    PERM[ii, ii + 1] = 1.0
    # pre-exp causal mask for one head pair: 0 where k<=q else -1e30
    TRINEG = np.where(np.arange(128)[:, None] <= np.arange(128)[None, :],
                      0.0, -1e30).astype(np.float32)
    inv_freq = (1.0 / (10000.0 ** (np.arange(0, H, 2, dtype=np.float32) / H)))
    sinus = pos[:, None] * inv_freq[None, :]              # [T, 32]
    cosT = np.tile(np.cos(sinus).T, (4, 1))               # [128, T]
    sinT = np.tile(np.sin(sinus).T, (4, 1))
    consts = np.concatenate(
        [cosT, sinT, IDENT, PERM, np.tile(TRINEG, (1, 2))], axis=1)
    consts = consts.astype(ml_dtypes.bfloat16)            # [128, 2560]

    in_maps = []
    for core in range(8):
        b, g = divmod(core, 4)
        hs = slice(g * 512, (g + 1) * 512)
        Wq = (w_attn[:, 0:2048][:, hs] * SCALE).astype(np.float32)
        Wk = w_attn[:, 2048:4096][:, hs]
        Wv = w_attn[:, 4096:6144][:, hs]
        WA = np.concatenate([Wq, Wk, Wv], axis=1)         # [2048, 1536]
        wAr = np.ascontiguousarray(
            WA.reshape(16, 128, 12, 128).transpose(1, 2, 0, 3)  # ki mi ko mc
        ).astype(ml_dtypes.bfloat16)
        wPr = np.ascontiguousarray(
            w_proj[hs, :].reshape(4, 128, 2048).transpose(1, 0, 2)
        ).astype(ml_dtypes.bfloat16)
        xTr = np.ascontiguousarray(
            x[b].T.reshape(16, 128, 1024).transpose(1, 0, 2)
        ).astype(ml_dtypes.bfloat16)
        in_maps.append({
            "xT": xTr, "wA": wAr, "wP": wPr, "consts": consts,
        })
    return in_maps


def kernel(x, pos, w_attn, w_proj, _trace=False):
    global _NC_CACHE
    _install_patches()
    from concourse.bass_utils import run_bass_kernel_spmd

    if _NC_CACHE is None:
        _NC_CACHE = _build_bass()
    nc = _NC_CACHE
    in_maps = _host_inputs(x, pos, w_attn, w_proj)
    res = run_bass_kernel_spmd(nc, in_maps, core_ids=list(range(8)), trace=_trace)
    outs = [res.results[c]["out"].astype(np.float32) for c in range(8)]
    full = np.stack([
        outs[0] + outs[1] + outs[2] + outs[3],
        outs[4] + outs[5] + outs[6] + outs[7],
    ]).astype(np.float32)
    kernel.last_results = res
    return full


# revision 46
# speedup vs baseline: 1.0133x; 1.0133x over previous
"""Causal self-attention (RoPE quirk variant) on 8 Trainium2 NeuronCores.

Reference computation (B=2, T=1024, C=2048, H=64 heads, hd=32):
  qkv = x @ w_attn; split q,k,v; RoPE(dim=n_head quirk) on q,k;
  causal softmax attention; y @ w_proj.

Sharding: 8 cores = 2 batches x 4 head-quarter shards (16 heads / core).
Each core computes attention for its 16 heads on its batch and a partial
output projection (its 512 channels of the 2048-channel contraction);
the host sums the 4 partials per batch.

Device layout notes:
  * Everything "transposed": qkv^T [chan, T] so the attention matmuls
    (contraction over hd on partitions) and the out-projection
    (contraction over channels on partitions) need no transposes.
  * All HBM traffic is bf16 (x, w_attn, w_proj, out) - halves DMA time;
    matmuls accumulate fp32 in PSUM so the overall error stays ~0.6%.
  * rotate_half is a signed permutation matmul on PE (a partition
    pair-swap DMA gets only half bandwidth at 64 partitions and its
    queue latency sat on the S critical path).
  * Scores are built as S^T [T_k, T_q]: 4 heads row-strip-packed, 2
    heads per 2-bank PSUM tile; the causal mask is a pre-exp -1e30 add
    on the PSUM scores (on the S->exp edge, hidden under ACT's queue,
    instead of the exp->PV critical edge); one exp per head pair on
    ACT; denominators via a ones-matrix matmul whose 4 col-strip
    matmuls pack with the PV quads (measured: col-strip quads start
    within 10 ns; row-strip S quads serialize on the PSUM write port);
    1/z = exp(-ln z) on ACT (bass blocks the Reciprocal LUT and the
    DVE reciprocal costs 6.4 ns/col).
  * Projections accumulate half-T chains into 2 double-buffered PSUM
    banks so DVE cast drains overlap the next chain (ACT stays
    dedicated to exp).
  * Schedule shape (everything is emission-order = in-order per
    engine): warm-up matmuls against the consts tile pre-release the
    HAM clock gate (idle default 1.2 GHz) before x lands; per group
    the big attention half's S/exp blocks are woven with the group's
    v-projection halves, and the small half's blocks are woven with
    the NEXT group's q/k projection chains, so stretches of 32-row S
    matmuls (which read as low activity to the clock gate) always
    carry full-row projection matmuls; g3's small half interleaves
    with out-projection units (which run on the by-then-idle psa
    slots) block by block, tq 4-7 first.
"""

import json
import sys

sys.path.insert(0, "/opt/trn_rl_repo")

import ml_dtypes
import numpy as np

import concourse.bass as bass
import concourse.mybir as mybir
import concourse.tile as tile
from concourse.tile import add_dep_helper

F32 = mybir.dt.float32
BF16 = mybir.dt.bfloat16

B, T, C = 2, 1024, 2048
H, HD = 64, 32
SCALE = 1.0 / np.sqrt(32.0)

_PATCHED = False


def _split_excess_waits(bir_json: bytes) -> bytes:
    """The walrus build in this container encodes at most ONE sync-wait per
    instruction; Tile's wait assigner emits several. Hoist excess waits onto
    same-engine NoOps placed immediately before the instruction."""
    d = json.loads(bir_json)
    ctr = 0
    for fn in d.get("functions", []):
        for blk in fn.get("blocks", []):
            out = []
            for inst in blk.get("instructions", []):
                si = inst.get("sync_info")
                waits = (si or {}).get("on_wait") or []
                if len(waits) > 1:
                    for w in waits[:-1]:
                        out.append({
                            "name": f"WSplit-{ctr}",
                            "opcode": "NoOp",
                            "engine": inst["engine"],
                            "ins": [],
                            "outs": [],
                            "sync_info": {"on_update": [], "on_wait": [w]},
                        })
                        ctr += 1
                    si["on_wait"] = [waits[-1]]
                out.append(inst)
            blk["instructions"] = out
    return json.dumps(d).encode()


def _install_patches():
    global _PATCHED
    if _PATCHED:
        return
    import concourse.bass_utils as bu
    import concourse.bass2jax as b2j

    orig = bu.compile_bir_kernel

    def patched_compile(bir_json, tmpdir, neff_name="file.neff"):
        return orig(_split_excess_waits(bir_json), tmpdir, neff_name)

    bu.compile_bir_kernel = patched_compile
    b2j.compile_bir_kernel = patched_compile
    _PATCHED = True


def _build_bass():
    nc = bass.Bass(trn_type="TRN2")
    xT = nc.dram_tensor("xT", [128, 16, 1024], BF16, kind="ExternalInput").ap()
    wA = nc.dram_tensor("wA", [128, 12, 16, 128], BF16, kind="ExternalInput").ap()
    wP = nc.dram_tensor("wP", [128, 4, 2048], BF16, kind="ExternalInput").ap()
    # consts: cos[1024] | sin[1024] | ident[128] | perm[128] | trineg[256]
    consts = nc.dram_tensor("consts", [128, 2560], BF16, kind="ExternalInput").ap()
    out = nc.dram_tensor("out", [1024, 2048], BF16, kind="ExternalOutput").ap()
    outr = out.rearrange("(tq p) n -> tq p n", p=128)

    EXP = mybir.ActivationFunctionType.Exp
    LN = mybir.ActivationFunctionType.Ln
    COPY = mybir.ActivationFunctionType.Copy

    with tile.TileContext(nc) as tc:
        with tc.tile_pool(name="persist", bufs=1) as persist, \
             tc.tile_pool(name="ylate", bufs=1) as ylate:
            # rotated q (dim1 = group 0-3) and k (4-7), bf16 [chan, T]
            qkT = persist.tile([128, 8, 1024], BF16)
            v_sb = persist.tile([128, 8, 512], BF16)    # [T_k in blk, kb, chan]
            csts = persist.tile([128, 2560], BF16)
            nc.scalar.dma_start(csts, consts)
            cos_sb = csts[:, 0:1024]
            sin_sb = csts[:, 1024:2048]
            sin2 = sin_sb.rearrange("p (a b) -> p a b", b=512)
            id_sb = csts[:, 2048:2176]
            perm_sb = csts[:, 2176:2304]                # signed rotate_half
            # [128, 2, 128]: 0 where k<=q else -1e30, for one head pair
            trineg = csts[:, 2304:2560].rearrange("p (a b) -> p a b", b=128)
            ones_sb = persist.tile([128, 32], BF16)
            nc.vector.memset(ones_sb, 1.0)
            wp_sb = persist.tile([128, 4, 2048], BF16)  # loaded at g==2
            y_tiles = [ylate.tile([128, 1024], BF16, tag=f"y{g}", name=f"y{g}")
                       for g in range(4)]

            with tc.tile_pool(name="phA", bufs=1) as xpool, \
                 tc.tile_pool(name="wstream", bufs=3) as wsp:
                xt = xpool.tile([128, 16, 1024], BF16)
                # prefetch the first weight chunk ahead of the x bulk so the
                # first projection matmul isn't stuck behind the x DMA.
                # The sync engine's bulk queue has the fastest ramp-up;
                # scalar/gpsimd bulk queues start ~3us later and drain
                # slower (measured), so everything big goes on sync.
                wt_first = wsp.tile([128, 16, 128], BF16, tag="wa", name="wt_first")
                nc.sync.dma_start(wt_first, wA[:, 0])
                for kc in range(8):
                    nc.sync.dma_start(xt[:, 2 * kc:2 * kc + 2, :],
                                      xT[:, 2 * kc:2 * kc + 2, :])

                # ---- per-group: project q,k -> RoPE -> project v -> attn ----
                with tc.tile_pool(name="esp", bufs=10) as esp, \
                     tc.tile_pool(name="qtp", bufs=2) as qtp, \
                     tc.tile_pool(name="zp", bufs=2) as zp, \
                     tc.tile_pool(name="phD", bufs=2) as phd, \
                     tc.tile_pool(name="psA", bufs=2, space="PSUM") as psa, \
                     tc.tile_pool(name="psS", bufs=2, space="PSUM") as psS, \
                     tc.tile_pool(name="psYZ", bufs=1, space="PSUM") as psYZ:

                    def attn_sblock(g, qc, kb):
                        # scores (+ pre-exp causal mask) + exp for one 128-k
                        # block; reads only qkT, so it may be emitted before
                        # the group's v projection. The mask is a -1e30 add
                        # on the PSUM scores: it sits on the S->exp edge
                        # (hidden under ACT's queue) instead of the exp->PV
                        # critical edge.
                        q0 = qc * 512
                        k0 = kb * 128
                        n0 = max(q0, k0)
                        N = q0 + 512 - n0
                        es = esp.tile([128, 4, 512], BF16, tag="es")
                        pss = [psS.tile([128, 2, 512], F32, tag="pss",
                                        name=f"pss{g}_{qc}_{kb}_{p}")
                               for p in range(2)]
                        for h in range(4):
                            nc.tensor.matmul(
                                pss[h // 2][:, h % 2, :N],
                                qkT[32 * h:32 * h + 32, 4 + g, k0:k0 + 128],
                                qkT[32 * h:32 * h + 32, g, n0:n0 + N],
                                start=True, stop=True,
                                tile_position=(32 * h, 0))
                        if k0 >= q0:
                            for p in range(2):
                                nc.vector.tensor_add(
                                    pss[p][:, :, 0:128], pss[p][:, :, 0:128],
                                    trineg)
                        nc.scalar.activation(
                            es[:, 0:2, :N], pss[0][:, :, :N], EXP)
                        e1 = nc.scalar.activation(
                            es[:, 2:4, :N], pss[1][:, :, :N], EXP)
                        return es, e1, N, n0 - q0

                    def attn_pvz(g, qc, kb, sblk, psy, psz):
                        es, gate, N, off = sblk
                        nkb = (qc + 1) * 4
                        first = True
                        for h in range(4):
                            c0 = g * 128 + 32 * h
                            pv = nc.tensor.matmul(
                                psy[32 * h:32 * h + 32, off:512],
                                v_sb[:, kb, c0:c0 + 32],
                                es[:, h, :N],
                                start=(kb == 0), stop=(kb == nkb - 1),
                                tile_position=(0, 32 * h),
                                skip_group_check=True)
                            if first:
                                add_dep_helper(pv.ins, gate.ins,
                                               sync=True, reason="pack PV")
                                first = False
                        first = True
                        for h in range(4):
                            z = nc.tensor.matmul(
                                psz[32 * h:32 * h + 32, off:512],
                                ones_sb,
                                es[:, h, :N],
                                start=(kb == 0), stop=(kb == nkb - 1),
                                tile_position=(0, 32 * h),
                                skip_group_check=True)
                            if first:
                                add_dep_helper(z.ins, gate.ins,
                                               sync=True, reason="pack Z")
                                first = False

                    def attn_block(g, qc, kb, psy, psz):
                        attn_pvz(g, qc, kb, attn_sblock(g, qc, kb), psy, psz)

                    def attn_finish(g, q0, psy, psz):
                        # 1/z = exp(-ln(z)) on ACT (reciprocal LUT is
                        # blocked in bass; DVE reciprocal costs 6.4ns/col)
                        zinv = zp.tile([128, 512], F32, tag="zinv")
                        lnz = zp.tile([128, 512], F32, tag="lnz")
                        nc.scalar.activation(lnz, psz, LN)
                        nc.scalar.activation(zinv, lnz, EXP, scale=-1.0)
                        nc.vector.tensor_mul(y_tiles[g][:, q0:q0 + 512],
                                             psy, zinv)

                    def outproj_tq(tq):
                        # 32 independent (tq, n) units on the psa slots - no
                        # dedicated out-projection PSUM pool, so these can be
                        # interleaved with tail attention blocks
                        o_sb = phd.tile([128, 2048], BF16, tag="osb")
                        for n in range(4):
                            pso = psa.tile([128, 512], F32, tag="ps",
                                           name=f"pso{tq}_{n}")
                            for gk in range(4):
                                nc.tensor.matmul(
                                    pso, y_tiles[gk][:, tq * 128:(tq + 1) * 128],
                                    wp_sb[:, gk, n * 512:(n + 1) * 512],
                                    start=(gk == 0), stop=(gk == 3))
                            sl = slice(n * 512, (n + 1) * 512)
                            if n % 2 == 0:
                                nc.vector.tensor_copy(o_sb[:, sl], pso)
                            else:
                                nc.scalar.activation(o_sb[:, sl], pso, COPY)
                        nc.sync.dma_start(outr[tq], o_sb)

                    def rope(g, pre, swp):
                        # rotate_half is a signed permutation matmul on PE (a
                        # partition pair-swap DMA only gets half bandwidth at
                        # 64 partitions and its queue latency sat on the S
                        # critical path)
                        for j, dst in enumerate((qkT[:, g, :], qkT[:, 4 + g, :])):
                            rot = psS.tile([128, 2, 512], F32, tag="pss",
                                           name=f"rot{g}_{j}")
                            for half in (0, 1):
                                nc.tensor.matmul(
                                    rot[:, half, :], perm_sb,
                                    pre[:, j, half * 512:half * 512 + 512],
                                    start=True, stop=True)
                            nc.vector.tensor_mul(
                                swp[:, j, :].rearrange("p (a b) -> p a b", b=512),
                                rot, sin2)
                            nc.vector.tensor_mul(pre[:, j, :], pre[:, j, :], cos_sb)
                            nc.vector.tensor_add(dst, pre[:, j, :], swp[:, j, :])

                    def qk_phase(g, blocks=()):
                        # project q,k then RoPE; `blocks` are attention-block
                        # thunks of the previous group's small half, woven
                        # between the projection half-chains so the PE keeps
                        # full-row matmul activity (the HAM clock gate
                        # throttles to 1.2 GHz through stretches of 32-row
                        # S matmuls) and the exp chain stalls get filled.
                        pre = qtp.tile([128, 2, 1024], BF16, tag="pre")
                        swp = qtp.tile([128, 2, 1024], BF16, tag="swp")
                        if g == 0:
                            wtq = wt_first
                        else:
                            wtq = wsp.tile([128, 16, 128], BF16, tag="wa")
                            nc.sync.dma_start(wtq, wA[:, g])
                        wtk = wsp.tile([128, 16, 128], BF16, tag="wa")
                        nc.sync.dma_start(wtk, wA[:, 4 + g])
                        seq = [(wtq, 0, pre[:, 0, :]), (wtq, 1, pre[:, 0, :]),
                               (wtk, 0, pre[:, 1, :]), (wtk, 1, pre[:, 1, :])]
                        for i, (wt, half, dst) in enumerate(seq):
                            if i < len(blocks):
                                blocks[i]()
                            proj_half(g, wt, half, dst)
                        rope(g, pre, swp)

                    def proj_half(g, wt, half, dst):
                        ps = psa.tile([128, 512], F32, tag="ps")
                        sl = slice(half * 512, half * 512 + 512)
                        for ko in range(16):
                            nc.tensor.matmul(ps, wt[:, ko, :], xt[:, ko, sl],
                                             start=(ko == 0), stop=(ko == 15))
                        nc.vector.tensor_copy(dst[:, sl], ps)

                    def vT_range(g, vtmp, kbs):
                        # natural-layout v transpose via PE; draws from the
                        # psa pool (proj-paced) so it never queues behind
                        # exp-gated S tiles in psS
                        for kb in kbs:
                            pst = psa.tile([128, 512], F32, tag="ps",
                                           name=f"pst{g}_{kb}")[:, 0:128]
                            nc.tensor.matmul(pst,
                                             vtmp[:, kb * 128:(kb + 1) * 128],
                                             id_sb, start=True, stop=True)
                            nc.vector.tensor_copy(
                                v_sb[:, kb, g * 128:(g + 1) * 128], pst)

                    def attn1_woven(g):
                        # the group's big attention half, software-pipelined
                        # against its v projection: S/exp blocks (which need
                        # only q,k) interleave with the v-projection matmuls
                        # so the ACT exp chains hide under PE work; PV/Z
                        # follow as packed bursts once v is transposed.
                        wt = wsp.tile([128, 16, 128], BF16, tag="wa")
                        nc.sync.dma_start(wt, wA[:, 8 + g])
                        vtmp = qtp.tile([128, 1024], BF16, tag="vtmp")
                        sb = {}
                        sb[0] = attn_sblock(g, 1, 0)
                        sb[1] = attn_sblock(g, 1, 1)
                        proj_half(g, wt, 0, vtmp)
                        sb[2] = attn_sblock(g, 1, 2)
                        sb[3] = attn_sblock(g, 1, 3)
                        proj_half(g, wt, 1, vtmp)
                        sb[4] = attn_sblock(g, 1, 4)
                        vT_range(g, vtmp, range(0, 4))
                        sb[5] = attn_sblock(g, 1, 5)
                        vT_range(g, vtmp, range(4, 8))
                        sb[6] = attn_sblock(g, 1, 6)
                        sb[7] = attn_sblock(g, 1, 7)
                        psy = psYZ.tile([128, 512], F32, tag="psy")
                        psz = psYZ.tile([128, 512], F32, tag="psz")
                        for kb in range(8):
                            attn_pvz(g, 1, kb, sb[kb], psy, psz)
                        attn_finish(g, 512, psy, psz)

                    qk_phase(0)
                    attn1_woven(0)
                    for g in (1, 2, 3):
                        # previous group's small half woven into this
                        # group's q/k projection chains
                        gp = g - 1
                        psy0 = psYZ.tile([128, 512], F32, tag="psy")
                        psz0 = psYZ.tile([128, 512], F32, tag="psz")
                        blocks = [
                            (lambda kb=kb: attn_block(gp, 0, kb, psy0, psz0))
                            for kb in range(4)]
                        qk_phase(g, blocks)
                        attn_finish(gp, 0, psy0, psz0)
                        if g == 2:
                            nc.scalar.dma_start(wp_sb, wP)
                        attn1_woven(g)

                    # ---- tail: g3 small half interleaved with
                    # out-projection units ----
                    psy30 = psYZ.tile([128, 512], F32, tag="psy")
                    psz30 = psYZ.tile([128, 512], F32, tag="psz")
                    for kb in range(4):
                        attn_block(3, 0, kb, psy30, psz30)
                        outproj_tq(4 + kb)
                    attn_finish(3, 0, psy30, psz30)
                    for tq in (0, 1, 2, 3):
                        outproj_tq(tq)
    return nc


_NC_CACHE = None


def _host_inputs(x, pos, w_attn, w_proj):
    """Build the 8 per-core input dicts (all device traffic in bf16)."""
    x = np.asarray(x, dtype=np.float32)
    pos = np.asarray(pos, dtype=np.float32)
    w_attn = np.asarray(w_attn, dtype=np.float32)
    w_proj = np.asarray(w_proj, dtype=np.float32)

    IDENT = np.eye(128, dtype=np.float32)
    # signed rotate_half permutation: rot[2i] = -x[2i+1], rot[2i+1] = x[2i]
    PERM = np.zeros((128, 128), dtype=np.float32)
    ii = np.arange(0, 128, 2)
    PERM[ii + 1, ii] = -1.0
    PERM[ii, ii + 1] = 1.0
    # pre-exp causal mask for one head pair: 0 where k<=q else -1e30
    TRINEG = np.where(np.arange(128)[:, None] <= np.arange(128)[None, :],
                      0.0, -1e30).astype(np.float32)
    inv_freq = (1.0 / (10000.0 ** (np.arange(0, H, 2, dtype=np.float32) / H)))
    sinus = pos[:, None] * inv_freq[None, :]              # [T, 32]
    cosT = np.tile(np.cos(sinus).T, (4, 1))               # [128, T]
    sinT = np.tile(np.sin(sinus).T, (4, 1))
    consts = np.concatenate(
        [cosT, sinT, IDENT, PERM, np.tile(TRINEG, (1, 2))], axis=1)
    consts = consts.astype(ml_dtypes.bfloat16)            # [128, 2560]

    in_maps = []
    for core in range(8):
        b, g = divmod(core, 4)
        hs = slice(g * 512, (g + 1) * 512)
        Wq = (w_attn[:, 0:2048][:, hs] * SCALE).astype(np.float32)
        Wk = w_attn[:, 2048:4096][:, hs]
        Wv = w_attn[:, 4096:6144][:, hs]
        WA = np.concatenate([Wq, Wk, Wv], axis=1)         # [2048, 1536]
        wAr = np.ascontiguousarray(
            WA.reshape(16, 128, 12, 128).transpose(1, 2, 0, 3)  # ki mi ko mc
        ).astype(ml_dtypes.bfloat16)
        wPr = np.ascontiguousarray(
            w_proj[hs, :].reshape(4, 128, 2048).transpose(1, 0, 2)
        ).astype(ml_dtypes.bfloat16)
        xTr = np.ascontiguousarray(
            x[b].T.reshape(16, 128, 1024).transpose(1, 0, 2)
        ).astype(ml_dtypes.bfloat16)
        in_maps.append({
            "xT": xTr, "wA": wAr, "wP": wPr, "consts": consts,
        })
    return in_maps


def kernel(x, pos, w_attn, w_proj, _trace=False):
    global _NC_CACHE
    _install_patches()
    from concourse.bass_utils import run_bass_kernel_spmd

    if _NC_CACHE is None:
        _NC_CACHE = _build_bass()
    nc = _NC_CACHE
    in_maps = _host_inputs(x, pos, w_attn, w_proj)
    res = run_bass_kernel_spmd(nc, in_maps, core_ids=list(range(8)), trace=_trace)
    outs = [res.results[c]["out"].astype(np.float32) for c in range(8)]
    full = np.stack([
        outs[0] + outs[1] + outs[2] + outs[3],
        outs[4] + outs[5] + outs[6] + outs[7],
    ]).astype(np.float32)
    kernel.last_results = res
    return full
